# revision 24
# baseline (speedup 1.0000x reference)
"""Trainium2 Bass kernel for the 1D differentiable Euler solver (Roe flux,
Harten entropy fix, CFL-adaptive dt, 32 first-order steps).

Strategy (8 NeuronCores, SPMD):
  - Spatial shard: 131,072 cells/core as [128 partitions x 1024 cells] plus
    G=32 edge-clamped ghost cells per partition side (G >= n_steps); whole
    time loop in SBUF. Compute windows shrink one cell per side per step
    (no ghost sanitization; only the masked global-edge clamp).
  - fp32 state + stage-A chain (keeps the per-step critical path pipelined);
    fp16 for the interface/dissipation chain (DVE 2x). Scale factors folded
    into ACT activation scales / tensor_scalar immediates.
  - Roe averages -> arithmetic averages (O(jump^2) perturbation of the
    dissipation only): removes the sqrt(rho)-weighted average machinery.
  - Engine split: DVE runs the fp16 tensor-tensor chain; ACT does casts,
    sqrt, square, abs; GPSIMD does central fluxes + flux differences and
    the dt AllReduce, overlapped with stage B.

kernel(**inputs) takes FULL unsharded inputs, returns full (rho, u, p).
"""

import numpy as np

import concourse.bass as bass
import concourse.bacc as bacc
import concourse.tile as tile
import concourse.mybir as mybir
from concourse import bass_isa
from concourse.bass_utils import run_bass_kernel_spmd

F32 = mybir.dt.float32
F16 = mybir.dt.float16
U8 = mybir.dt.uint8
ALU = mybir.AluOpType
ACTF = mybir.ActivationFunctionType
AX = mybir.AxisListType

GAMMA = 1.4
CFL = 0.5
DX = 1e-3

NX = 1048576
NC = 8
P = 128
FPC = NX // NC // P
G = 32
W = FPC + 2 * G
V = W - 1

_CACHE = {}
_last_results = None


def _build(n_steps: int):
    nc = bacc.Bacc("TRN2", target_bir_lowering=False, debug=False,
                   enable_asserts=False, num_devices=NC)

    rho_in = nc.dram_tensor("rho_in", [P, W], F32, kind="ExternalInput")
    mu_in = nc.dram_tensor("mu_in", [P, W], F32, kind="ExternalInput")
    E_in = nc.dram_tensor("E_in", [P, W], F32, kind="ExternalInput")
    tf_in = nc.dram_tensor("tf_in", [1, 1], F32, kind="ExternalInput")
    mskL_in = nc.dram_tensor("mskL_in", [P, G], U8, kind="ExternalInput")
    mskR_in = nc.dram_tensor("mskR_in", [P, G], U8, kind="ExternalInput")
    rho_out = nc.dram_tensor("rho_out", [P, FPC], F32, kind="ExternalOutput")
    u_out = nc.dram_tensor("u_out", [P, FPC], F32, kind="ExternalOutput")
    p_out = nc.dram_tensor("p_out", [P, FPC], F32, kind="ExternalOutput")

    with tile.TileContext(nc) as tc:
        with (
            tc.tile_pool(name="sb", bufs=1) as sb,
            tc.tile_pool(name="dram", bufs=1, space="DRAM") as dram,
        ):
            rho = sb.tile([P, W], F32, tag="rho", name="rho")
            mu = sb.tile([P, W], F32, tag="mu", name="mu")
            En = sb.tile([P, W], F32, tag="En", name="En")

            # fp32 work tiles
            N32 = 8
            wk32 = [sb.tile([P, W], F32, tag=f"w32_{i}", name=f"w32_{i}")
                    for i in range(N32)]
            f32free = list(wk32)
            f32live = {}

            def g32(name):
                t = f32free.pop()
                f32live[name] = t
                return t

            def r32(*names):
                for n in names:
                    f32free.append(f32live.pop(n))

            # fp16 work tiles
            N16 = 36
            wk16 = [sb.tile([P, W], F16, tag=f"w16_{i}", name=f"w16_{i}")
                    for i in range(N16)]
            free = list(wk16)
            live = {}

            def get(name):
                t = free.pop()
                live[name] = t
                return t

            def rel(*names):
                for n in names:
                    free.append(live.pop(n))

            mskL = sb.tile([P, G], U8, tag="mskL", name="mskL")
            mskR = sb.tile([P, G], U8, tag="mskR", name="mskR")
            small = {}
            for n in ("wmax", "gpp", "gball", "rgi", "rgs", "dt0", "rem",
                      "dtt", "tcur", "hdtn", "tfb", "cE", "cF"):
                small[n] = sb.tile([P, 1], F32, tag=n, name=n)
            tf1 = sb.tile([1, 1], F32, tag="tf1", name="tf1")

            cc_in = dram.tile([P, 1], F32, tag="cc_in", name="cc_in")
            cc_out = dram.tile([P, 1], F32, tag="cc_out", name="cc_out")

            vec = nc.vector
            act = nc.scalar
            gps = nc.gpsimd

            # ---- prologue ----
            nc.sync.dma_start(out=rho[:], in_=rho_in.ap())
            nc.sync.dma_start(out=mu[:], in_=mu_in.ap())
            nc.sync.dma_start(out=En[:], in_=E_in.ap())
            nc.sync.dma_start(out=mskL[:], in_=mskL_in.ap())
            nc.sync.dma_start(out=mskR[:], in_=mskR_in.ap())
            nc.sync.dma_start(out=tf1[:], in_=tf_in.ap())
            gps.partition_broadcast(small["tfb"][:], tf1[:])
            vec.memset(small["tcur"][:], 0.0)
            vec.memset(small["cE"][:], 0.71428573)   # de' = dSp + 0.7143*w2
            vec.memset(small["cF"][:], -0.7)         # gE = dcE - 0.7*dde'

            for s in range(n_steps):
                vW = V
                u0 = 1
                uW = W - 2

                def A(t):
                    return t[:, 0:W]

                def Li(t):
                    return t[:, 0:V]

                def Ri(t):
                    return t[:, 1:W]

                def I(t):
                    return t[:, 0:vW]

                if s > 0:
                    # refill stale columns from nearest valid column, then
                    # re-clamp global-edge ghosts (masked)
                    for st in (rho, mu, En):
                        act.copy(st[:, 0:s], st[:, s:s + 1].broadcast_to((P, s)))
                        act.copy(st[:, W - s:W],
                                 st[:, W - s - 1:W - s].broadcast_to((P, s)))
                    for st in (rho, mu, En):
                        vec.copy_predicated(st[:, 0:G], mskL[:],
                                            st[:, G:G + 1].broadcast_to((P, G)))
                        vec.copy_predicated(st[:, W - G:W], mskR[:],
                                            st[:, W - G - 1:W - G].broadcast_to((P, G)))

                # ---- stage A (fp32 chain on vec; casts on act) ----
                rinv = g32("rinv")
                vec.reciprocal_approx_fast(A(rinv), A(rho))
                uu = g32("uu")
                vec.tensor_tensor(A(uu), A(mu), A(rinv), ALU.mult)
                q = g32("q")
                vec.tensor_tensor(A(q), A(mu), A(uu), ALU.mult)
                p04 = g32("p04")
                vec.scalar_tensor_tensor(A(p04), A(q), -0.5, A(En),
                                         ALU.mult, ALU.add)
                pp = g32("pp")
                act.mul(A(pp), A(p04), 0.4)

                rh25 = get("rh25")
                act.mul(A(rh25), A(rho), 2.5)
                au16 = get("au16")
                act.activation(A(au16), A(uu), ACTF.Abs)

                # gps: central fluxes + differences
                dcr = get("dcr")
                gps.tensor_tensor(dcr[:, 0:uW], mu[:, u0 + 1:u0 + 1 + uW],
                                  mu[:, u0 - 1:u0 - 1 + uW], ALU.subtract)
                Ep = g32("Ep")
                gps.tensor_tensor(A(Ep), A(En), A(pp), ALU.add)
                Fm = g32("Fm")
                gps.tensor_tensor(A(Fm), A(q), A(pp), ALU.add)
                r32("q")
                Fe = g32("Fe")
                gps.tensor_tensor(A(Fe), A(uu), A(Ep), ALU.mult)
                r32("Ep")
                dcm = get("dcm")
                gps.tensor_tensor(dcm[:, 0:uW], Fm[:, u0 + 1:u0 + 1 + uW],
                                  Fm[:, u0 - 1:u0 - 1 + uW], ALU.subtract)
                r32("Fm")
                dce = get("dce")
                gps.tensor_tensor(dce[:, 0:uW], Fe[:, u0 + 1:u0 + 1 + uW],
                                  Fe[:, u0 - 1:u0 - 1 + uW], ALU.subtract)
                r32("Fe")

                # ---- fp16 feeders + stage B ----
                # pr25 = p04/rho = 2.5*p/rho (fp32 in -> fp16 out)
                pr25 = get("pr25")
                vec.tensor_tensor(A(pr25), A(p04), A(rinv), ALU.mult)
                r32("rinv")
                cc16 = get("cc16")
                act.activation(A(cc16), A(pr25), ACTF.Sqrt, scale=0.56)

                # S = pr25L + pr25R ; cbar2 = 0.28*S
                S16 = get("S16")
                vec.tensor_tensor(I(S16), Li(pr25), Ri(pr25), ALU.add)
                rel("pr25")
                S32 = g32("S32")
                act.copy(I(S32), I(S16))
                u2b = get("u2b")
                vec.tensor_tensor(I(u2b), Li(uu), Ri(uu), ALU.add)
                ur = get("ur")
                vec.tensor_scalar(I(ur), I(u2b), 0.5, None, ALU.mult)
                rel("u2b")
                du = get("du")
                vec.tensor_tensor(I(du), Ri(uu), Li(uu), ALU.subtract)
                dpd = get("dpd")
                vec.tensor_tensor(I(dpd), Ri(p04), Li(p04), ALU.subtract)
                r32("p04")
                drd = get("drd")
                vec.tensor_tensor(I(drd), Ri(rho), Li(rho), ALU.subtract)
                crdu = get("crdu")
                vec.tensor_tensor(I(crdu), Ri(rh25), I(du), ALU.mult)
                rel("du", "rh25")

                # wave-speed max + collective kickoff
                wsc = get("wsc")
                own = slice(G, G + FPC)
                vec.tensor_tensor(wsc[:, own], au16[:, own], cc16[:, own],
                                  ALU.add)
                rel("au16", "cc16")
                vec.tensor_reduce(small["wmax"][:], wsc[:, own],
                                  axis=AX.X, op=ALU.max)
                rel("wsc")
                nc.sync.dma_start(out=cc_in[:], in_=small["wmax"][:])
                gps.collective_compute(
                    "AllReduce", ALU.max,
                    replica_groups=[list(range(NC))],
                    ins=[cc_in[:]], outs=[cc_out[:]])
                nc.sync.dma_start(out=small["gpp"][:], in_=cc_out[:])

                # reciprocal of S (fp32), scaled back to fp16
                rdS = g32("rdS")
                vec.reciprocal_approx_fast(I(rdS), I(S32))
                r32("S32")
                rd16 = get("rd16")
                act.mul(I(rd16), I(rdS), 1.7857143)   # = 1/(2*cbar2)
                r32("rdS")
                cr16 = get("cr16")
                act.activation(I(cr16), I(S16), ACTF.Sqrt, scale=0.28)

                vec.tensor_tensor(I(crdu), I(crdu), I(cr16), ALU.mult)
                l1 = get("l1")
                vec.tensor_tensor(I(l1), I(ur), I(cr16), ALU.subtract)
                l3 = get("l3")
                vec.tensor_tensor(I(l3), I(ur), I(cr16), ALU.add)
                ur2f = get("ur2f")
                act.activation(I(ur2f), I(ur), ACTF.Square)
                s1 = get("s1")
                act.activation(I(s1), I(l1), ACTF.Square)
                rel("l1")
                s3 = get("s3")
                act.activation(I(s3), I(l3), ACTF.Square)
                rel("l3")
                e2 = get("e2")
                vec.tensor_scalar(I(e2), I(S16), 0.0028, None, ALU.mult)
                vec.tensor_tensor(I(s1), I(s1), I(e2), ALU.add)
                vec.tensor_tensor(I(s3), I(s3), I(e2), ALU.add)
                a2t = get("a2t")
                vec.tensor_tensor(I(a2t), I(ur2f), I(e2), ALU.add)
                rel("ur2f", "e2")
                a1s = get("a1s")
                act.activation(I(a1s), I(s1), ACTF.Sqrt, scale=0.16)
                rel("s1")
                a3s = get("a3s")
                act.activation(I(a3s), I(s3), ACTF.Sqrt, scale=0.16)
                rel("s3")
                a2s = get("a2s")
                act.activation(I(a2s), I(a2t), ACTF.Sqrt)
                rel("a2t")

                X1 = get("X1")
                vec.tensor_tensor(I(X1), I(dpd), I(crdu), ALU.subtract)
                X3 = get("X3")
                vec.tensor_tensor(I(X3), I(dpd), I(crdu), ALU.add)
                rel("crdu")
                rdX = get("rdX")
                vec.tensor_scalar(I(rdX), I(rd16), 0.8, None, ALU.mult)
                mtt = get("mtt")
                vec.tensor_tensor(I(mtt), I(dpd), I(rdX), ALU.mult)
                rel("dpd", "rdX")
                M2 = get("M2")
                vec.tensor_tensor(I(M2), I(drd), I(mtt), ALU.subtract)
                rel("drd", "mtt")
                vec.tensor_tensor(I(X1), I(a1s), I(X1), ALU.mult)
                rel("a1s")
                vec.tensor_tensor(I(X3), I(a3s), I(X3), ALU.mult)
                rel("a3s")
                bp = get("bp")
                vec.tensor_tensor(I(bp), I(X1), I(X3), ALU.add)
                bm = get("bm")
                vec.tensor_tensor(I(bm), I(X3), I(X1), ALU.subtract)
                rel("X1", "X3")
                G2 = get("G2")
                vec.tensor_tensor(I(G2), I(a2s), I(M2), ALU.mult)
                rel("a2s", "M2")
                Sp = get("Sp")
                vec.tensor_tensor(I(Sp), I(bp), I(rd16), ALU.mult)
                rel("bp")
                Sm = get("Sm")
                vec.tensor_tensor(I(Sm), I(bm), I(rd16), ALU.mult)
                rel("bm", "rd16")
                dr = get("dr")
                vec.tensor_tensor(I(dr), I(Sp), I(G2), ALU.add)
                rel("G2")
                csm = get("csm")
                vec.tensor_tensor(I(csm), I(cr16), I(Sm), ALU.mult)
                rel("cr16", "Sm")
                dm = get("dm")
                vec.tensor_tensor(I(dm), I(ur), I(dr), ALU.mult)
                vec.tensor_tensor(I(dm), I(dm), I(csm), ALU.add)
                dSp = get("dSp")
                vec.tensor_tensor(I(dSp), I(S16), I(Sp), ALU.mult)
                rel("S16", "Sp")
                w2 = get("w2")
                vec.tensor_tensor(I(w2), I(dm), I(csm), ALU.add)
                rel("csm")
                vec.tensor_tensor(I(w2), I(ur), I(w2), ALU.mult)
                rel("ur")
                deE = get("deE")
                vec.scalar_tensor_tensor(I(deE), I(w2), small["cE"][:],
                                         I(dSp), ALU.mult, ALU.add)
                rel("w2", "dSp")

                # ---- dt chain ----
                gps.partition_all_reduce(small["gball"][:], small["gpp"][:],
                                         channels=P,
                                         reduce_op=bass_isa.ReduceOp.max)
                vec.reciprocal_approx_accurate(small["rgi"][:],
                                               small["gball"][:],
                                               small["rgs"][:])
                vec.tensor_scalar_mul(small["dt0"][:], small["rgi"][:],
                                      float(CFL * DX))
                vec.scalar_tensor_tensor(small["rem"][:], small["tcur"][:],
                                         -1.0, small["tfb"][:],
                                         ALU.mult, ALU.add)
                vec.tensor_scalar_max(small["rem"][:], small["rem"][:], 0.0)
                vec.tensor_tensor(small["dtt"][:], small["dt0"][:],
                                  small["rem"][:], ALU.min)
                vec.tensor_tensor(small["tcur"][:], small["tcur"][:],
                                  small["dtt"][:], ALU.add)
                vec.tensor_scalar_mul(small["hdtn"][:], small["dtt"][:],
                                      float(-0.5 / DX))

                # ---- updates: st += hdtn*(dcF - dd) ----
                ddr = get("ddr")
                vec.tensor_tensor(ddr[:, 0:uW], dr[:, 1:1 + uW],
                                  dr[:, 0:uW], ALU.subtract)
                rel("dr")
                ddm = get("ddm")
                vec.tensor_tensor(ddm[:, 0:uW], dm[:, 1:1 + uW],
                                  dm[:, 0:uW], ALU.subtract)
                rel("dm")
                dde = get("dde")
                vec.tensor_tensor(dde[:, 0:uW], deE[:, 1:1 + uW],
                                  deE[:, 0:uW], ALU.subtract)
                rel("deE")

                gR = live["dcr"]
                vec.tensor_tensor(gR[:, 0:uW], gR[:, 0:uW], ddr[:, 0:uW],
                                  ALU.subtract)
                vec.scalar_tensor_tensor(rho[:, u0:u0 + uW], gR[:, 0:uW],
                                         small["hdtn"][:], rho[:, u0:u0 + uW],
                                         ALU.mult, ALU.add)
                rel("dcr", "ddr")
                gM = live["dcm"]
                vec.tensor_tensor(gM[:, 0:uW], gM[:, 0:uW], ddm[:, 0:uW],
                                  ALU.subtract)
                vec.scalar_tensor_tensor(mu[:, u0:u0 + uW], gM[:, 0:uW],
                                         small["hdtn"][:], mu[:, u0:u0 + uW],
                                         ALU.mult, ALU.add)
                rel("dcm", "ddm")
                gE = live["dce"]
                vec.scalar_tensor_tensor(gE[:, 0:uW], dde[:, 0:uW],
                                         small["cF"][:], gE[:, 0:uW],
                                         ALU.mult, ALU.add)
                vec.scalar_tensor_tensor(En[:, u0:u0 + uW], gE[:, 0:uW],
                                         small["hdtn"][:], En[:, u0:u0 + uW],
                                         ALU.mult, ALU.add)
                rel("dce", "dde")
                r32("uu", "pp")
                assert len(free) == N16, (s, len(free), sorted(live))
                assert len(f32free) == N32, (s, len(f32free), sorted(f32live))

            # ---- epilogue ----
            own = slice(G, G + FPC)
            w1, w2_, w3 = g32("e1"), g32("e2"), g32("e3")
            vec.reciprocal_approx_fast(w1[:, own], rho[:, own])
            vec.tensor_tensor(w2_[:, own], mu[:, own], w1[:, own], ALU.mult)
            vec.tensor_tensor(w3[:, own], mu[:, own], w2_[:, own], ALU.mult)
            vec.scalar_tensor_tensor(w3[:, own], w3[:, own], -0.5,
                                     En[:, own], ALU.mult, ALU.add)
            act.mul(w3[:, own], w3[:, own], 0.4)
            nc.sync.dma_start(out=rho_out.ap(), in_=rho[:, own])
            nc.sync.dma_start(out=u_out.ap(), in_=w2_[:, own])
            nc.sync.dma_start(out=p_out.ap(), in_=w3[:, own])

    nc.compile()
    return nc


def _get_program(n_steps: int):
    if n_steps not in _CACHE:
        _CACHE[n_steps] = _build(n_steps)
    return _CACHE[n_steps]


def kernel(rho_init, u_init, p_init, t_final, n_steps):
    rho_init = np.ascontiguousarray(np.asarray(rho_init, np.float32))
    u_init = np.ascontiguousarray(np.asarray(u_init, np.float32))
    p_init = np.ascontiguousarray(np.asarray(p_init, np.float32))
    tf = np.float32(np.asarray(t_final).reshape(()))
    ns = int(np.asarray(n_steps).reshape(()))
    assert rho_init.shape == (NX,)
    assert ns <= G

    gm1 = np.float32(GAMMA - 1.0)
    cells = NX // NC
    idx = (np.arange(P)[:, None] * FPC) + (np.arange(W)[None, :] - G)

    in_maps = []
    for k in range(NC):
        gi = np.clip(k * cells + idx, 0, NX - 1)
        r = rho_init[gi]
        u = u_init[gi]
        p = p_init[gi]
        mu_ = r * u
        E = p / gm1 + np.float32(0.5) * r * u * u
        mskL = np.zeros((P, G), np.uint8)
        mskR = np.zeros((P, G), np.uint8)
        if k == 0:
            mskL[0, :] = 1
        if k == NC - 1:
            mskR[P - 1, :] = 1
        in_maps.append({
            "rho_in": np.ascontiguousarray(r),
            "mu_in": np.ascontiguousarray(mu_),
            "E_in": np.ascontiguousarray(E),
            "tf_in": np.full((1, 1), tf, np.float32),
            "mskL_in": mskL,
            "mskR_in": mskR,
        })

    nc = _get_program(ns)
    res = run_bass_kernel_spmd(nc, in_maps, core_ids=list(range(NC)))
    global _last_results
    _last_results = res

    rho_o = np.empty(NX, np.float32)
    u_o = np.empty(NX, np.float32)
    p_o = np.empty(NX, np.float32)
    for k in range(NC):
        sl = slice(k * cells, (k + 1) * cells)
        rho_o[sl] = res.results[k]["rho_out"].reshape(-1)
        u_o[sl] = res.results[k]["u_out"].reshape(-1)
        p_o[sl] = res.results[k]["p_out"].reshape(-1)
    return rho_o, u_o, p_o
